# revision 8
# baseline (speedup 1.0000x reference)
"""Trainium2 Bass kernel for nn_CorrProductBlock — sorted-by-element design.

Nodes are sorted by element id on the HOST and each element's segment is
padded to a multiple of 256.  Every 256-node half-tile is then single-element,
so the per-element symmetric-contraction weights become per-partition SCALAR
columns — the five one-hot gather matmuls and their five DVE PSUM-rate
consumers of the previous design collapse into four tensor_scalar /
scalar_tensor_tensor minis, and the one-hot stream disappears from HBM.

Features travel channel-major bf16 both ways (host pre/post transpose).
Node-parallel across 8 NeuronCores; self-contained.
"""

import numpy as np
import ml_dtypes

import concourse.bass as bass
import concourse.bacc as bacc
import concourse.mybir as mybir
import concourse.tile as tile
from concourse.bass_utils import run_bass_kernel_spmd

MUL = 128
NUM_ELEM = 64
N_CORES = 8
TILE_N = 512
HALF = 256

F32 = mybir.dt.float32
BF16 = mybir.dt.bfloat16

MULT = mybir.AluOpType.mult
ADD = mybir.AluOpType.add


def _build(ntiles: int, repeat: int = 1):
    nc = bacc.Bacc(num_devices=N_CORES, dynamic_dma_scratch_size=98304)

    xt = nc.dram_tensor("xt", [128, ntiles, 4, TILE_N], BF16, kind="ExternalInput")
    # per-(tile, half) weight columns: 5 tables (t10,t11,t200,t211*s3,t201)
    wc = nc.dram_tensor("wc", [128, ntiles, 2, 5], F32, kind="ExternalInput")
    wpre0 = nc.dram_tensor("wpre0", [MUL, MUL], BF16, kind="ExternalInput")
    wpre1 = nc.dram_tensor("wpre1", [MUL, MUL], BF16, kind="ExternalInput")
    wco0 = nc.dram_tensor("wco0", [MUL, MUL], BF16, kind="ExternalInput")
    wco1 = nc.dram_tensor("wco1", [MUL, MUL], BF16, kind="ExternalInput")
    wsc0 = nc.dram_tensor("wsc0", [MUL, MUL], BF16, kind="ExternalInput")
    wsc1 = nc.dram_tensor("wsc1", [MUL, MUL], BF16, kind="ExternalInput")
    yt = nc.dram_tensor("yt", [128, ntiles, 4, TILE_N], BF16, kind="ExternalOutput")

    with tile.TileContext(nc) as tc:
        with (
            tc.tile_pool(name="singles", bufs=1) as singles,
            tc.tile_pool(name="xin", bufs=5) as xin_pool,
            tc.tile_pool(name="ew", bufs=4) as ew_pool,
            tc.tile_pool(name="outp", bufs=3) as out_pool,
            tc.tile_pool(name="ph", bufs=1, space="PSUM") as ph_pool,
            tc.tile_pool(name="pu", bufs=2, space="PSUM") as pu_pool,
        ):
            def load_w(dram, p, tag):
                t = singles.tile([p, MUL], BF16, tag=tag)
                nc.sync.dma_start(out=t, in_=dram[:, :])
                return t

            W_pre0 = load_w(wpre0, 128, "wpre0")
            W_pre1 = load_w(wpre1, 128, "wpre1")
            W_co0 = load_w(wco0, 128, "wco0")
            W_co1 = load_w(wco1, 128, "wco1")
            W_sc0 = load_w(wsc0, 128, "wsc0")
            W_sc1 = load_w(wsc1, 128, "wsc1")
            wc_t = singles.tile([128, ntiles, 2, 5], F32, tag="wc")
            nc.sync.dma_start(out=wc_t, in_=wc[:, :, :, :])

            import contextlib
            # unroll the pipeline twice per For_i iteration: the two passes
            # share one scheduling region (deep cross-pass overlap), so the
            # loop-seam ramp cost is paid half as often. repeat semantics
            # are preserved (repeat = number of full-pipeline passes).
            if repeat > 1 and repeat % 4 == 0:
                n_body, n_iter = 4, repeat // 4
            elif repeat > 1 and repeat % 2 == 0:
                n_body, n_iter = 2, repeat // 2
            else:
                n_body, n_iter = 1, repeat
            rep_ctx = (
                tc.For_i(0, n_iter, hint_engines=tuple(mybir.ALL_ENGINES))
                if n_iter > 1 else contextlib.nullcontext()
            )
            with rep_ctx:
                for _ in range(n_body):
                    _tile_body(nc, tc, locals())

    nc.compile()
    return nc


def _tile_body(nc, tc, env):
    """Software-pipelined emission (engine streams are FIFO in program order)."""
    ntiles = env["ntiles"]
    xt, yt, wc_t = env["xt"], env["yt"], env["wc_t"]
    xin_pool, ew_pool, out_pool = env["xin_pool"], env["ew_pool"], env["out_pool"]
    ph_pool, pu_pool = env["ph_pool"], env["pu_pool"]
    W_pre0, W_pre1 = env["W_pre0"], env["W_pre1"]
    W_co0, W_co1 = env["W_co0"], env["W_co1"]
    W_sc0, W_sc1 = env["W_sc0"], env["W_sc1"]

    st = [dict() for _ in range(ntiles)]

    def stage_load(i):
        if not (0 <= i < ntiles):
            return
        xT = xin_pool.tile([128, 4, TILE_N], BF16, tag="xT")
        nc.sync.dma_start(out=xT, in_=xt[:, i])
        st[i]["xT"] = xT

    def stage_pre(i):
        # h in two 2-bank psum tiles so the h-evac loop pipelines at half-tile
        # granularity (ph bufs=3 holds 1.5 tile-generations in 6 banks)
        if not (0 <= i < ntiles):
            return
        xT = st[i]["xT"]
        h = ph_pool.tile([128, 4, TILE_N], F32, tag="h")
        nc.tensor.matmul(h[:, 0, :], W_pre0, xT[:, 0, :], start=True, stop=True)
        for i3 in range(3):
            nc.tensor.matmul(
                h[:, 1 + i3, :], W_pre1, xT[:, 1 + i3, :], start=True, stop=True
            )
        st[i]["h"] = h

    def stage_evac(i):
        if not (0 <= i < ntiles):
            return
        hc = ew_pool.tile([128, 4, TILE_N], BF16, tag="hc")
        nc.scalar.copy(out=hc, in_=st[i]["h"])
        st[i]["c0"], st[i]["c1"] = hc[:, 0, :], hc[:, 1:4, :]

    def stage_m(i):
        # m_i = (c1_i * w211col) * c1_i ; the sum over the three components
        # rides the PSUM accumulation of u0 (three extra co-matmuls).
        if not (0 <= i < ntiles):
            return
        c1 = st[i]["c1"]
        sq = ew_pool.tile([128, 3, TILE_N], BF16, tag="sq")
        nc.vector.tensor_mul(sq, c1, c1)
        m = ew_pool.tile([128, 3, TILE_N], BF16, tag="m")
        for j in range(2):
            sl = slice(j * HALF, (j + 1) * HALF)
            wcb = wc_t[:, i, j, :]
            nc.vector.tensor_scalar(
                out=m[:, :, sl], in0=sq[:, :, sl],
                scalar1=wcb[:, 3:4], scalar2=None, op0=MULT,
            )
        st[i]["m"] = m

    def stage_ts(i):
        # per-half tensor_scalar minis: t2 = c0*w200 + w10 ; p2 = c0*w201 + w11
        if not (0 <= i < ntiles):
            return
        c0 = st[i]["c0"]
        t2 = ew_pool.tile([128, TILE_N], BF16, tag="t2")
        p2 = ew_pool.tile([128, TILE_N], BF16, tag="p2")
        for j in range(2):
            sl = slice(j * HALF, (j + 1) * HALF)
            wcb = wc_t[:, i, j, :]
            nc.vector.tensor_scalar(
                out=t2[:, sl], in0=c0[:, sl],
                scalar1=wcb[:, 2:3], scalar2=wcb[:, 0:1], op0=MULT, op1=ADD,
            )
            nc.vector.tensor_scalar(
                out=p2[:, sl], in0=c0[:, sl],
                scalar1=wcb[:, 4:5], scalar2=wcb[:, 1:2], op0=MULT, op1=ADD,
            )
        st[i]["t2"], st[i]["p2"] = t2, p2

    def stage_comb(i):
        if not (0 <= i < ntiles):
            return
        c0, c1 = st[i]["c0"], st[i]["c1"]
        t2, p2 = st[i]["t2"], st[i]["p2"]
        a0a = ew_pool.tile([128, TILE_N], BF16, tag="a0a")
        nc.gpsimd.tensor_tensor(out=a0a, in0=c0, in1=t2, op=MULT)
        a1 = ew_pool.tile([128, 3, TILE_N], BF16, tag="a1")
        p2b = bass.AP(
            tensor=p2.tensor, offset=p2.offset,
            ap=[p2.ap[0], [0, 3], p2.ap[1]],
        )
        nc.vector.tensor_tensor(out=a1, in0=p2b, in1=c1, op=MULT)
        st[i]["a0a"], st[i]["a1"] = a0a, a1

    def stage_final_mm(i):
        if not (0 <= i < ntiles):
            return
        xT, a1 = st[i]["xT"], st[i]["a1"]
        a0a, m = st[i]["a0a"], st[i]["m"]
        out_sb = out_pool.tile([128, 4, TILE_N], BF16, tag="out")
        u01 = pu_pool.tile([128, 2, TILE_N], F32, tag="u")
        nc.tensor.matmul(u01[:, 0, :], W_sc0, xT[:, 0, :], start=True, stop=False)
        nc.tensor.matmul(u01[:, 0, :], W_co0, a0a, start=False, stop=False)
        for i3 in range(3):
            nc.tensor.matmul(u01[:, 0, :], W_co0, m[:, i3, :],
                             start=False, stop=(i3 == 2))
        nc.tensor.matmul(u01[:, 1, :], W_sc1, xT[:, 1, :], start=True, stop=False)
        nc.tensor.matmul(u01[:, 1, :], W_co1, a1[:, 0, :], start=False, stop=True)
        nc.scalar.copy(out=out_sb[:, 0:2, :], in_=u01)
        u23 = pu_pool.tile([128, 2, TILE_N], F32, tag="u")
        for k in (2, 3):
            nc.tensor.matmul(
                u23[:, k - 2, :], W_sc1, xT[:, k, :], start=True, stop=False)
            nc.tensor.matmul(
                u23[:, k - 2, :], W_co1, a1[:, k - 1, :], start=False, stop=True)
        nc.scalar.copy(out=out_sb[:, 2, :], in_=u23[:, 0, :])
        st[i]["out_sb"], st[i]["u23"] = out_sb, u23

    def stage_ucopy(i):
        if not (0 <= i < ntiles):
            return
        nc.vector.tensor_copy(out=st[i]["out_sb"][:, 3, :], in_=st[i]["u23"][:, 1, :])

    def stage_out(i):
        if not (0 <= i < ntiles):
            return
        nc.sync.dma_start(out=yt[:, i], in_=st[i]["out_sb"])
        st[i].clear()

    stage_load(0)
    stage_load(1)
    stage_pre(0)
    # 2-tile stagger on the final matmuls: the elementwise chain latency of
    # tile i is hidden because FM(i) runs two loop iterations later.
    for i in range(ntiles + 2):
        stage_load(i + 2)
        stage_evac(i)
        stage_final_mm(i - 2)
        stage_m(i)
        stage_ts(i)
        stage_comb(i)
        stage_ucopy(i - 2)
        stage_out(i - 2)
        stage_pre(i + 1)


def _prep_weights(inp):
    s = 1.0 / np.sqrt(MUL)
    f = lambda a: np.asarray(a, dtype=np.float32)
    bf = lambda a: np.ascontiguousarray(a.astype(ml_dtypes.bfloat16))
    w = {}
    w["wpre0"] = bf(f(inp["Wpre0"]) * s)
    w["wpre1"] = bf(f(inp["Wpre1"]) * s)
    w["wco0"] = bf((f(inp["Wprod0"]) @ f(inp["Wout0"])) * (s * s))
    w["wco1"] = bf((f(inp["Wprod1"]) @ f(inp["Wout1"])) * (s * s))
    w["wsc0"] = bf(f(inp["Wsc0"]) * s)
    w["wsc1"] = bf(f(inp["Wsc1"]) * s)
    return w


def _tables(inp):
    """[5, 64, 128] f32: t10, t11, t200, t211*inv_sqrt3, t201."""
    s3 = 1.0 / np.sqrt(3.0)
    f = lambda a: np.asarray(a, dtype=np.float32)
    return np.stack([
        f(inp["w1_0"]), f(inp["w1_1"]), f(inp["w2_00"]),
        f(inp["w2_11"]) * s3, f(inp["w2_01"]),
    ])


_cache = {}


def _get_program(ntiles):
    if ntiles not in _cache:
        _cache[ntiles] = _build(ntiles)
    return _cache[ntiles]


def _shard_feats(block, ntiles):
    """[per, 512] f32 -> channel-major bf16 [128, ntiles, 4, TILE_N]."""
    per = block.shape[0]
    out = np.empty((128, per, 4), dtype=ml_dtypes.bfloat16)
    out[:, :, 0] = block[:, :MUL].T
    v = block[:, MUL:].reshape(per, MUL, 3)
    out[:, :, 1:4] = np.moveaxis(v, 0, 1)
    out = out.reshape(128, ntiles, TILE_N, 4).transpose(0, 1, 3, 2)
    return np.ascontiguousarray(out)


def kernel(**inputs):
    inputs = {k: np.asarray(v) for k, v in inputs.items()}
    node_feats = inputs["node_feats"].astype(np.float32, copy=False)
    elems = np.asarray(inputs["node_elems"]).astype(np.int64)
    n = node_feats.shape[0]
    weights = _prep_weights(inputs)
    T5 = _tables(inputs)

    # sort by element, pad each segment to a multiple of HALF
    order = np.argsort(elems, kind="stable")
    cnt = np.bincount(elems, minlength=NUM_ELEM)
    segpad = ((cnt + HALF - 1) // HALF) * HALF
    starts = np.zeros(NUM_ELEM, np.int64)
    starts[1:] = np.cumsum(segpad)[:-1]
    # position of each sorted node inside the padded layout
    seg_begin = np.zeros(NUM_ELEM, np.int64)
    seg_begin[1:] = np.cumsum(cnt)[:-1]
    within = np.arange(n) - np.repeat(seg_begin, cnt)
    pos = np.repeat(starts, cnt) + within

    n_halves = int(segpad.sum()) // HALF
    halves_elem = np.repeat(np.arange(NUM_ELEM), segpad // HALF)
    n_tiles = (n_halves + 1) // 2
    n_tiles = ((n_tiles + N_CORES - 1) // N_CORES) * N_CORES
    ntiles = n_tiles // N_CORES
    tot = n_tiles * TILE_N
    halves_elem = np.concatenate(
        [halves_elem, np.zeros(n_tiles * 2 - n_halves, np.int64)]
    )

    xpad = np.zeros((tot, 512), dtype=np.float32)
    xpad[pos] = node_feats[order]
    wc_all = T5[:, halves_elem, :]                     # [5, n_tiles*2, 128]
    wc_all = wc_all.reshape(5, n_tiles, 2, 128).transpose(3, 1, 2, 0)
    wc_all = np.ascontiguousarray(wc_all.astype(np.float32))

    per_core = ntiles * TILE_N
    in_maps = []
    for c in range(N_CORES):
        blk = xpad[c * per_core:(c + 1) * per_core]
        xtc = _shard_feats(blk, ntiles)
        wcc = np.ascontiguousarray(
            wc_all[:, c * ntiles:(c + 1) * ntiles]
        )
        in_maps.append({"xt": xtc, "wc": wcc, **weights})

    nc = _get_program(ntiles)
    res = run_bass_kernel_spmd(nc, in_maps, core_ids=list(range(N_CORES)))

    ypad = np.empty((tot, 512), dtype=np.float32)
    for c in range(N_CORES):
        yc = res.results[c]["yt"].astype(np.float32)
        yc = yc.transpose(1, 3, 2, 0).reshape(per_core, 4, 128)
        lo = c * per_core
        ypad[lo:lo + per_core, :MUL] = yc[:, 0]
        ypad[lo:lo + per_core, MUL:] = (
            yc[:, 1:4].transpose(0, 2, 1).reshape(per_core, 384)
        )
    out = np.empty((n, 512), dtype=np.float32)
    out[order] = ypad[pos]
    return out


def build_bench(ntiles, repeat):
    """Program + synthetic sorted inputs for slope timing."""
    per_core = ntiles * TILE_N
    nc = _build(ntiles, repeat=repeat)
    rng = np.random.default_rng(0)
    bf = lambda shape: rng.standard_normal(shape, dtype=np.float32).astype(
        ml_dtypes.bfloat16
    )
    w = {
        "wpre0": bf((MUL, MUL)), "wpre1": bf((MUL, MUL)),
        "wco0": bf((MUL, MUL)), "wco1": bf((MUL, MUL)),
        "wsc0": bf((MUL, MUL)), "wsc1": bf((MUL, MUL)),
    }
    xtc = bf((128, ntiles, 4, TILE_N))
    wcc = rng.standard_normal((128, ntiles, 2, 5), dtype=np.float32)
    in_maps = [{"xt": xtc, "wc": wcc, **w} for _ in range(N_CORES)]
    return nc, in_maps


# expected tile count for the 100k/64-element benchmark distribution
BENCH_NTILES = 27


# revision 9
# speedup vs baseline: 1.1602x; 1.1602x over previous
"""Trainium2 Bass kernel for nn_CorrProductBlock — sorted-by-element design.

Nodes are sorted by element id on the HOST and each element's segment is
padded to a multiple of 256.  Every 256-node half-tile is then single-element,
so the per-element symmetric-contraction weights become per-partition SCALAR
columns — the five one-hot gather matmuls and their five DVE PSUM-rate
consumers of the previous design collapse into four tensor_scalar /
scalar_tensor_tensor minis, and the one-hot stream disappears from HBM.

Features travel channel-major bf16 both ways (host pre/post transpose).
Node-parallel across 8 NeuronCores; self-contained.
"""

import numpy as np
import ml_dtypes

import concourse.bass as bass
import concourse.bacc as bacc
import concourse.mybir as mybir
import concourse.tile as tile
from concourse.bass_utils import run_bass_kernel_spmd

MUL = 128
NUM_ELEM = 64
N_CORES = 8
TILE_N = 512
HALF = 256

F32 = mybir.dt.float32
BF16 = mybir.dt.bfloat16

MULT = mybir.AluOpType.mult
ADD = mybir.AluOpType.add


def _build(ntiles: int, repeat: int = 1):
    nc = bacc.Bacc(num_devices=N_CORES, dynamic_dma_scratch_size=98304)

    xt = nc.dram_tensor("xt", [128, ntiles, 4, TILE_N], BF16, kind="ExternalInput")
    # per-(tile, half) weight columns: 5 tables (t10,t11,t200,t211*s3,t201)
    wc = nc.dram_tensor("wc", [128, ntiles, 2, 5], F32, kind="ExternalInput")
    wpre0 = nc.dram_tensor("wpre0", [MUL, MUL], BF16, kind="ExternalInput")
    wpre1 = nc.dram_tensor("wpre1", [MUL, MUL], BF16, kind="ExternalInput")
    wco0 = nc.dram_tensor("wco0", [MUL, MUL], BF16, kind="ExternalInput")
    wco1 = nc.dram_tensor("wco1", [MUL, MUL], BF16, kind="ExternalInput")
    wsc0 = nc.dram_tensor("wsc0", [MUL, MUL], BF16, kind="ExternalInput")
    wsc1 = nc.dram_tensor("wsc1", [MUL, MUL], BF16, kind="ExternalInput")
    yt = nc.dram_tensor("yt", [128, ntiles, 4, TILE_N], BF16, kind="ExternalOutput")

    with tile.TileContext(nc) as tc:
        with (
            tc.tile_pool(name="singles", bufs=1) as singles,
            tc.tile_pool(name="xin", bufs=5) as xin_pool,
            tc.tile_pool(name="ew", bufs=4) as ew_pool,
            tc.tile_pool(name="outp", bufs=3) as out_pool,
            tc.tile_pool(name="ph", bufs=1, space="PSUM") as ph_pool,
            tc.tile_pool(name="pu", bufs=2, space="PSUM") as pu_pool,
        ):
            def load_w(dram, p, tag):
                t = singles.tile([p, MUL], BF16, tag=tag)
                nc.sync.dma_start(out=t, in_=dram[:, :])
                return t

            W_pre0 = load_w(wpre0, 128, "wpre0")
            W_pre1 = load_w(wpre1, 128, "wpre1")
            W_co0 = load_w(wco0, 128, "wco0")
            W_co1 = load_w(wco1, 128, "wco1")
            W_sc0 = load_w(wsc0, 128, "wsc0")
            W_sc1 = load_w(wsc1, 128, "wsc1")
            wc_t = singles.tile([128, ntiles, 2, 5], F32, tag="wc")
            nc.sync.dma_start(out=wc_t, in_=wc[:, :, :, :])

            import contextlib
            # unroll the pipeline twice per For_i iteration: the two passes
            # share one scheduling region (deep cross-pass overlap), so the
            # loop-seam ramp cost is paid half as often. repeat semantics
            # are preserved (repeat = number of full-pipeline passes).
            if repeat > 1 and repeat % 2 == 0:
                n_body, n_iter = 2, repeat // 2
            else:
                n_body, n_iter = 1, repeat
            rep_ctx = (
                tc.For_i(0, n_iter, hint_engines=tuple(mybir.ALL_ENGINES))
                if n_iter > 1 else contextlib.nullcontext()
            )
            with rep_ctx:
                for _ in range(n_body):
                    _tile_body(nc, tc, locals())

    nc.compile()
    return nc


def _tile_body(nc, tc, env):
    """Software-pipelined emission (engine streams are FIFO in program order)."""
    ntiles = env["ntiles"]
    xt, yt, wc_t = env["xt"], env["yt"], env["wc_t"]
    xin_pool, ew_pool, out_pool = env["xin_pool"], env["ew_pool"], env["out_pool"]
    ph_pool, pu_pool = env["ph_pool"], env["pu_pool"]
    W_pre0, W_pre1 = env["W_pre0"], env["W_pre1"]
    W_co0, W_co1 = env["W_co0"], env["W_co1"]
    W_sc0, W_sc1 = env["W_sc0"], env["W_sc1"]

    st = [dict() for _ in range(ntiles)]

    def stage_load(i):
        if not (0 <= i < ntiles):
            return
        xT = xin_pool.tile([128, 4, TILE_N], BF16, tag="xT")
        nc.sync.dma_start(out=xT, in_=xt[:, i])
        st[i]["xT"] = xT

    def stage_pre(i):
        # h in two 2-bank psum tiles so the h-evac loop pipelines at half-tile
        # granularity (ph bufs=3 holds 1.5 tile-generations in 6 banks)
        if not (0 <= i < ntiles):
            return
        xT = st[i]["xT"]
        h = ph_pool.tile([128, 4, TILE_N], F32, tag="h")
        nc.tensor.matmul(h[:, 0, :], W_pre0, xT[:, 0, :], start=True, stop=True)
        for i3 in range(3):
            nc.tensor.matmul(
                h[:, 1 + i3, :], W_pre1, xT[:, 1 + i3, :], start=True, stop=True
            )
        st[i]["h"] = h

    def stage_evac(i):
        if not (0 <= i < ntiles):
            return
        hc = ew_pool.tile([128, 4, TILE_N], BF16, tag="hc")
        nc.scalar.copy(out=hc, in_=st[i]["h"])
        st[i]["c0"], st[i]["c1"] = hc[:, 0, :], hc[:, 1:4, :]

    def stage_m(i):
        # m_i = (c1_i * w211col) * c1_i ; the sum over the three components
        # rides the PSUM accumulation of u0 (three extra co-matmuls).
        if not (0 <= i < ntiles):
            return
        c1 = st[i]["c1"]
        sq = ew_pool.tile([128, 3, TILE_N], BF16, tag="sq")
        nc.vector.tensor_mul(sq, c1, c1)
        m = ew_pool.tile([128, 3, TILE_N], BF16, tag="m")
        for j in range(2):
            sl = slice(j * HALF, (j + 1) * HALF)
            wcb = wc_t[:, i, j, :]
            nc.vector.tensor_scalar(
                out=m[:, :, sl], in0=sq[:, :, sl],
                scalar1=wcb[:, 3:4], scalar2=None, op0=MULT,
            )
        st[i]["m"] = m

    def stage_ts(i):
        # per-half tensor_scalar minis: t2 = c0*w200 + w10 ; p2 = c0*w201 + w11
        if not (0 <= i < ntiles):
            return
        c0 = st[i]["c0"]
        t2 = ew_pool.tile([128, TILE_N], BF16, tag="t2")
        p2 = ew_pool.tile([128, TILE_N], BF16, tag="p2")
        for j in range(2):
            sl = slice(j * HALF, (j + 1) * HALF)
            wcb = wc_t[:, i, j, :]
            nc.vector.tensor_scalar(
                out=t2[:, sl], in0=c0[:, sl],
                scalar1=wcb[:, 2:3], scalar2=wcb[:, 0:1], op0=MULT, op1=ADD,
            )
            nc.vector.tensor_scalar(
                out=p2[:, sl], in0=c0[:, sl],
                scalar1=wcb[:, 4:5], scalar2=wcb[:, 1:2], op0=MULT, op1=ADD,
            )
        st[i]["t2"], st[i]["p2"] = t2, p2

    def stage_comb(i):
        if not (0 <= i < ntiles):
            return
        c0, c1 = st[i]["c0"], st[i]["c1"]
        t2, p2 = st[i]["t2"], st[i]["p2"]
        a0a = ew_pool.tile([128, TILE_N], BF16, tag="a0a")
        nc.gpsimd.tensor_tensor(out=a0a, in0=c0, in1=t2, op=MULT)
        a1 = ew_pool.tile([128, 3, TILE_N], BF16, tag="a1")
        p2b = bass.AP(
            tensor=p2.tensor, offset=p2.offset,
            ap=[p2.ap[0], [0, 3], p2.ap[1]],
        )
        nc.vector.tensor_tensor(out=a1, in0=p2b, in1=c1, op=MULT)
        st[i]["a0a"], st[i]["a1"] = a0a, a1

    def stage_final_mm(i):
        if not (0 <= i < ntiles):
            return
        xT, a1 = st[i]["xT"], st[i]["a1"]
        a0a, m = st[i]["a0a"], st[i]["m"]
        out_sb = out_pool.tile([128, 4, TILE_N], BF16, tag="out")
        u01 = pu_pool.tile([128, 2, TILE_N], F32, tag="u")
        nc.tensor.matmul(u01[:, 0, :], W_sc0, xT[:, 0, :], start=True, stop=False)
        nc.tensor.matmul(u01[:, 0, :], W_co0, a0a, start=False, stop=False)
        for i3 in range(3):
            nc.tensor.matmul(u01[:, 0, :], W_co0, m[:, i3, :],
                             start=False, stop=(i3 == 2))
        nc.tensor.matmul(u01[:, 1, :], W_sc1, xT[:, 1, :], start=True, stop=False)
        nc.tensor.matmul(u01[:, 1, :], W_co1, a1[:, 0, :], start=False, stop=True)
        nc.scalar.copy(out=out_sb[:, 0:2, :], in_=u01)
        u23 = pu_pool.tile([128, 2, TILE_N], F32, tag="u")
        for k in (2, 3):
            nc.tensor.matmul(
                u23[:, k - 2, :], W_sc1, xT[:, k, :], start=True, stop=False)
            nc.tensor.matmul(
                u23[:, k - 2, :], W_co1, a1[:, k - 1, :], start=False, stop=True)
        nc.scalar.copy(out=out_sb[:, 2, :], in_=u23[:, 0, :])
        st[i]["out_sb"], st[i]["u23"] = out_sb, u23

    def stage_ucopy(i):
        if not (0 <= i < ntiles):
            return
        nc.vector.tensor_copy(out=st[i]["out_sb"][:, 3, :], in_=st[i]["u23"][:, 1, :])

    def stage_out(i):
        if not (0 <= i < ntiles):
            return
        nc.sync.dma_start(out=yt[:, i], in_=st[i]["out_sb"])
        st[i].clear()

    stage_load(0)
    stage_load(1)
    stage_pre(0)
    # 2-tile stagger on the final matmuls: the elementwise chain latency of
    # tile i is hidden because FM(i) runs two loop iterations later.
    for i in range(ntiles + 2):
        stage_load(i + 2)
        stage_evac(i)
        stage_final_mm(i - 2)
        stage_m(i)
        stage_ts(i)
        stage_comb(i)
        stage_ucopy(i - 2)
        stage_out(i - 2)
        stage_pre(i + 1)


def _prep_weights(inp):
    s = 1.0 / np.sqrt(MUL)
    f = lambda a: np.asarray(a, dtype=np.float32)
    bf = lambda a: np.ascontiguousarray(a.astype(ml_dtypes.bfloat16))
    w = {}
    w["wpre0"] = bf(f(inp["Wpre0"]) * s)
    w["wpre1"] = bf(f(inp["Wpre1"]) * s)
    w["wco0"] = bf((f(inp["Wprod0"]) @ f(inp["Wout0"])) * (s * s))
    w["wco1"] = bf((f(inp["Wprod1"]) @ f(inp["Wout1"])) * (s * s))
    w["wsc0"] = bf(f(inp["Wsc0"]) * s)
    w["wsc1"] = bf(f(inp["Wsc1"]) * s)
    return w


def _tables(inp):
    """[5, 64, 128] f32: t10, t11, t200, t211*inv_sqrt3, t201."""
    s3 = 1.0 / np.sqrt(3.0)
    f = lambda a: np.asarray(a, dtype=np.float32)
    return np.stack([
        f(inp["w1_0"]), f(inp["w1_1"]), f(inp["w2_00"]),
        f(inp["w2_11"]) * s3, f(inp["w2_01"]),
    ])


_cache = {}


def _get_program(ntiles):
    if ntiles not in _cache:
        _cache[ntiles] = _build(ntiles)
    return _cache[ntiles]


def _shard_feats(block, ntiles):
    """[per, 512] f32 -> channel-major bf16 [128, ntiles, 4, TILE_N]."""
    per = block.shape[0]
    out = np.empty((128, per, 4), dtype=ml_dtypes.bfloat16)
    out[:, :, 0] = block[:, :MUL].T
    v = block[:, MUL:].reshape(per, MUL, 3)
    out[:, :, 1:4] = np.moveaxis(v, 0, 1)
    out = out.reshape(128, ntiles, TILE_N, 4).transpose(0, 1, 3, 2)
    return np.ascontiguousarray(out)


def kernel(**inputs):
    inputs = {k: np.asarray(v) for k, v in inputs.items()}
    node_feats = inputs["node_feats"].astype(np.float32, copy=False)
    elems = np.asarray(inputs["node_elems"]).astype(np.int64)
    n = node_feats.shape[0]
    weights = _prep_weights(inputs)
    T5 = _tables(inputs)

    # sort by element, pad each segment to a multiple of HALF
    order = np.argsort(elems, kind="stable")
    cnt = np.bincount(elems, minlength=NUM_ELEM)
    segpad = ((cnt + HALF - 1) // HALF) * HALF
    starts = np.zeros(NUM_ELEM, np.int64)
    starts[1:] = np.cumsum(segpad)[:-1]
    # position of each sorted node inside the padded layout
    seg_begin = np.zeros(NUM_ELEM, np.int64)
    seg_begin[1:] = np.cumsum(cnt)[:-1]
    within = np.arange(n) - np.repeat(seg_begin, cnt)
    pos = np.repeat(starts, cnt) + within

    n_halves = int(segpad.sum()) // HALF
    halves_elem = np.repeat(np.arange(NUM_ELEM), segpad // HALF)
    n_tiles = (n_halves + 1) // 2
    n_tiles = ((n_tiles + N_CORES - 1) // N_CORES) * N_CORES
    ntiles = n_tiles // N_CORES
    tot = n_tiles * TILE_N
    halves_elem = np.concatenate(
        [halves_elem, np.zeros(n_tiles * 2 - n_halves, np.int64)]
    )

    xpad = np.zeros((tot, 512), dtype=np.float32)
    xpad[pos] = node_feats[order]
    wc_all = T5[:, halves_elem, :]                     # [5, n_tiles*2, 128]
    wc_all = wc_all.reshape(5, n_tiles, 2, 128).transpose(3, 1, 2, 0)
    wc_all = np.ascontiguousarray(wc_all.astype(np.float32))

    per_core = ntiles * TILE_N
    in_maps = []
    for c in range(N_CORES):
        blk = xpad[c * per_core:(c + 1) * per_core]
        xtc = _shard_feats(blk, ntiles)
        wcc = np.ascontiguousarray(
            wc_all[:, c * ntiles:(c + 1) * ntiles]
        )
        in_maps.append({"xt": xtc, "wc": wcc, **weights})

    nc = _get_program(ntiles)
    res = run_bass_kernel_spmd(nc, in_maps, core_ids=list(range(N_CORES)))

    ypad = np.empty((tot, 512), dtype=np.float32)
    for c in range(N_CORES):
        yc = res.results[c]["yt"].astype(np.float32)
        yc = yc.transpose(1, 3, 2, 0).reshape(per_core, 4, 128)
        lo = c * per_core
        ypad[lo:lo + per_core, :MUL] = yc[:, 0]
        ypad[lo:lo + per_core, MUL:] = (
            yc[:, 1:4].transpose(0, 2, 1).reshape(per_core, 384)
        )
    out = np.empty((n, 512), dtype=np.float32)
    out[order] = ypad[pos]
    return out


def build_bench(ntiles, repeat):
    """Program + synthetic sorted inputs for slope timing."""
    per_core = ntiles * TILE_N
    nc = _build(ntiles, repeat=repeat)
    rng = np.random.default_rng(0)
    bf = lambda shape: rng.standard_normal(shape, dtype=np.float32).astype(
        ml_dtypes.bfloat16
    )
    w = {
        "wpre0": bf((MUL, MUL)), "wpre1": bf((MUL, MUL)),
        "wco0": bf((MUL, MUL)), "wco1": bf((MUL, MUL)),
        "wsc0": bf((MUL, MUL)), "wsc1": bf((MUL, MUL)),
    }
    xtc = bf((128, ntiles, 4, TILE_N))
    wcc = rng.standard_normal((128, ntiles, 2, 5), dtype=np.float32)
    in_maps = [{"xt": xtc, "wc": wcc, **w} for _ in range(N_CORES)]
    return nc, in_maps


# expected tile count for the 100k/64-element benchmark distribution
BENCH_NTILES = 27
